# revision 1
# baseline (speedup 1.0000x reference)
"""Trainium2 Bass kernel for nn_NonSpikingOutput.

Reference semantics (N=4096 neurons, O=3 outputs, T=4096 steps):
    g_max = k/(e-k); act = clip(u, 0, 1); i_syn = g_max*act*(e - v)
    RK2 with i_syn frozen collapses to the per-element linear recurrence
        v_t = A_t * v_{t-1} + 0.075 * c_t * e_t,   A_t = 0.625 - 0.075*c_t
        c_t = act_t * k_t / (e_t - k_t)
    out[o, t] = sum_n v[n, o, t]

We scan w = v/0.075 (w_t = A_t*w + c_t*e_t) per (n,o) lane with the DVE
tensor_tensor_scan, and reduce over neurons with a PE matmul against a
(128,1) constant 0.075 tile, so out = 0.075 * colsum(w).

Sharding: neuron dim N split across 8 cores (512 neurons each); host sums
the per-core (O, T) partials.
"""

import sys
from contextlib import ExitStack

import numpy as np

sys.path.insert(0, "/opt/trn_rl_repo")

import concourse.bass as bass
import concourse.tile as tile
from concourse import bacc, mybir
from concourse.bass_utils import run_bass_kernel_spmd

N_CORES = 8
N, O, T = 4096, 3, 4096
NL = N // N_CORES  # neurons per core
NG = NL // 128     # 128-partition neuron groups per core
F = 2048           # time-chunk (free dim) per tile
TC = T // F
FP32 = mybir.dt.float32
OP = mybir.AluOpType
AF = mybir.ActivationFunctionType


def _build_nc() -> bass.Bass:
    nc = bacc.Bacc(
        "TRN2", target_bir_lowering=False, debug=False, num_devices=N_CORES
    )
    u = nc.dram_tensor("u", [NL, T], FP32, kind="ExternalInput")
    k = nc.dram_tensor("k", [NL, O, T], FP32, kind="ExternalInput")
    e = nc.dram_tensor("e", [NL, O, T], FP32, kind="ExternalInput")
    out = nc.dram_tensor("out", [O, T], FP32, kind="ExternalOutput")

    with tile.TileContext(nc) as tc, ExitStack() as ctx:
        # Preload the one ACT table set holding Ln+Exp+Copy (set 6,
        # natural_log_exp_and_others); otherwise the table chooser thrashes
        # between the Ln-only and Exp-only sets on every tile (~2.7us each).
        preload = mybir.InstLoadActFuncSet(
            name=nc.get_next_instruction_name(), act_func_set_id=6, ins=[], outs=[]
        )
        nc.scalar.add_instruction(preload)

        const_pool = ctx.enter_context(tc.tile_pool(name="const", bufs=1))
        scale = const_pool.tile([128, 1], FP32)
        nc.vector.memset(scale[:], 0.075)
        # one carry column per (o, g): column o*NG+g
        carry = const_pool.tile([128, O * NG], FP32)

        u_pool = ctx.enter_context(tc.tile_pool(name="u", bufs=NG + 1))
        k_pool = ctx.enter_context(tc.tile_pool(name="k", bufs=2))
        e_pool = ctx.enter_context(tc.tile_pool(name="e", bufs=2))
        d_pool = ctx.enter_context(tc.tile_pool(name="d", bufs=2))
        h_pool = ctx.enter_context(tc.tile_pool(name="h", bufs=2))
        t_pool = ctx.enter_context(tc.tile_pool(name="t", bufs=2))
        c_pool = ctx.enter_context(tc.tile_pool(name="c", bufs=2))
        a_pool = ctx.enter_context(tc.tile_pool(name="a", bufs=2))
        b_pool = ctx.enter_context(tc.tile_pool(name="b", bufs=2))
        w_pool = ctx.enter_context(tc.tile_pool(name="w", bufs=2))
        r_pool = ctx.enter_context(tc.tile_pool(name="r", bufs=2))
        ps_pool = ctx.enter_context(tc.tile_pool(name="ps", bufs=2, space="PSUM"))

        acts: dict[int, object] = {}

        for tci in range(TC):
            t0 = tci * F
            for o in range(O):
                ps = ps_pool.tile([1, F], FP32, tag="ps", name=f"ps{tci}_{o}")
                for g in range(NG):
                    p0 = g * 128
                    if o == 0:
                        ut = u_pool.tile([128, F], FP32, tag="u")
                        nc.sync.dma_start(ut[:], u[p0 : p0 + 128, t0 : t0 + F])
                        # act = clip(u, 0, 1), in place
                        nc.vector.tensor_scalar(ut[:], ut[:], 0.0, 1.0, OP.max, OP.min)
                        acts[g] = ut
                    act = acts[g]

                    kt = k_pool.tile([128, F], FP32, tag="k")
                    nc.sync.dma_start(kt[:], k[p0 : p0 + 128, o, t0 : t0 + F])
                    et = e_pool.tile([128, F], FP32, tag="e")
                    nc.sync.dma_start(et[:], e[p0 : p0 + 128, o, t0 : t0 + F])

                    dt = d_pool.tile([128, F], FP32, tag="d")
                    nc.gpsimd.tensor_tensor(dt[:], et[:], kt[:], OP.subtract)
                    # h = 1/d = exp(-ln(d)); d is in (1, 3) so this is accurate
                    ht = h_pool.tile([128, F], FP32, tag="h")
                    nc.scalar.activation(ht[:], dt[:], AF.Ln)
                    nc.scalar.activation(ht[:], ht[:], AF.Exp, scale=-1.0)
                    tt = t_pool.tile([128, F], FP32, tag="t")
                    nc.vector.tensor_tensor(tt[:], kt[:], ht[:], OP.mult)
                    ct = c_pool.tile([128, F], FP32, tag="c")
                    nc.vector.tensor_tensor(ct[:], tt[:], act[:], OP.mult)
                    at = a_pool.tile([128, F], FP32, tag="a")
                    nc.vector.tensor_scalar(at[:], ct[:], -0.075, 0.625, OP.mult, OP.add)
                    bt = b_pool.tile([128, F], FP32, tag="b")
                    nc.vector.tensor_tensor(bt[:], ct[:], et[:], OP.mult)

                    wt = w_pool.tile([128, F], FP32, tag="w")
                    ci = o * NG + g
                    init = 0.0 if tci == 0 else carry[:, ci : ci + 1]
                    nc.vector.tensor_tensor_scan(wt[:], at[:], bt[:], init, OP.mult, OP.add)
                    if tci < TC - 1:
                        nc.scalar.copy(carry[:, ci : ci + 1], wt[:, F - 1 : F])

                    for s in range(F // 512):
                        nc.tensor.matmul(
                            ps[0:1, s * 512 : (s + 1) * 512],
                            scale[:],
                            wt[:, s * 512 : (s + 1) * 512],
                            start=(g == 0),
                            stop=(g == NG - 1),
                        )
                row = r_pool.tile([1, F], FP32, tag="row")
                nc.scalar.copy(row[:], ps[:])
                nc.sync.dma_start(out[o : o + 1, t0 : t0 + F], row[:, :])

    nc.compile()
    return nc


def _build_nc_v2() -> bass.Bass:
    """v2: engine-rebalanced + bf16 intermediates.

    Per (o, g, tchunk) tile of (128, F):
      GPSIMD: d = e - k (f32); kb = k cast to bf16
      ACT:    ln_d = Ln(d); h = Exp(-ln_d) -> bf16; A = -0.075*c + 0.625 -> bf16
              psum evacuation with the 0.075 output scale folded in
      DVE:    act = clip(u,0,1) -> bf16 (shared over o); t = kb*h (bf16 2x);
              c = t*act (bf16 2x); B = c*e (mixed, bf16 out); scan(A,B) -> w bf16
      PE:     ones^T @ w accumulated over the 4 neuron groups -> psum (1, F)
    """
    BF16 = mybir.dt.bfloat16
    nc = bacc.Bacc(
        "TRN2", target_bir_lowering=False, debug=False, num_devices=N_CORES
    )
    u = nc.dram_tensor("u", [NL, T], FP32, kind="ExternalInput")
    k = nc.dram_tensor("k", [NL, O, T], FP32, kind="ExternalInput")
    e = nc.dram_tensor("e", [NL, O, T], FP32, kind="ExternalInput")
    out = nc.dram_tensor("out", [O, T], FP32, kind="ExternalOutput")

    with tile.TileContext(nc) as tc, ExitStack() as ctx:
        preload = mybir.InstLoadActFuncSet(
            name=nc.get_next_instruction_name(), act_func_set_id=6, ins=[], outs=[]
        )
        nc.scalar.add_instruction(preload)

        const_pool = ctx.enter_context(tc.tile_pool(name="const", bufs=1))
        ones = const_pool.tile([128, 1], FP32)
        nc.vector.memset(ones[:], 1.0)
        carry = const_pool.tile([128, O * NG], FP32)

        u_pool = ctx.enter_context(tc.tile_pool(name="u", bufs=2))
        act_pool = ctx.enter_context(tc.tile_pool(name="act", bufs=NG + 1))
        k_pool = ctx.enter_context(tc.tile_pool(name="k", bufs=2))
        kb_pool = ctx.enter_context(tc.tile_pool(name="kb", bufs=2))
        e_pool = ctx.enter_context(tc.tile_pool(name="e", bufs=2))
        d_pool = ctx.enter_context(tc.tile_pool(name="d", bufs=2))
        h_pool = ctx.enter_context(tc.tile_pool(name="h", bufs=2))
        t_pool = ctx.enter_context(tc.tile_pool(name="t", bufs=2))
        c_pool = ctx.enter_context(tc.tile_pool(name="c", bufs=2))
        a_pool = ctx.enter_context(tc.tile_pool(name="a", bufs=2))
        b_pool = ctx.enter_context(tc.tile_pool(name="b", bufs=2))
        w_pool = ctx.enter_context(tc.tile_pool(name="w", bufs=2))
        r_pool = ctx.enter_context(tc.tile_pool(name="r", bufs=2))
        ps_pool = ctx.enter_context(tc.tile_pool(name="ps", bufs=2, space="PSUM"))

        acts: dict[int, object] = {}

        for tci in range(TC):
            t0 = tci * F
            for o in range(O):
                ps = ps_pool.tile([1, F], FP32, tag="ps", name=f"ps{tci}_{o}")
                for g in range(NG):
                    p0 = g * 128
                    if o == 0:
                        ut = u_pool.tile([128, F], FP32, tag="u")
                        nc.sync.dma_start(ut[:], u[p0 : p0 + 128, t0 : t0 + F])
                        av = act_pool.tile([128, F], BF16, tag="act")
                        nc.vector.tensor_scalar(av[:], ut[:], 0.0, 1.0, OP.max, OP.min)
                        acts[g] = av
                    act = acts[g]

                    kt = k_pool.tile([128, F], FP32, tag="k")
                    nc.sync.dma_start(kt[:], k[p0 : p0 + 128, o, t0 : t0 + F])
                    et = e_pool.tile([128, F], FP32, tag="e")
                    nc.sync.dma_start(et[:], e[p0 : p0 + 128, o, t0 : t0 + F])

                    dt = d_pool.tile([128, F], FP32, tag="d")
                    nc.gpsimd.tensor_tensor(dt[:], et[:], kt[:], OP.subtract)
                    kb = kb_pool.tile([128, F], BF16, tag="kb")
                    nc.gpsimd.tensor_scalar(kb[:], kt[:], 1.0, None, OP.mult)

                    ht = h_pool.tile([128, F], BF16, tag="h")
                    lnd = h_pool.tile([128, F], FP32, tag="lnd")
                    nc.scalar.activation(lnd[:], dt[:], AF.Ln)
                    nc.scalar.activation(ht[:], lnd[:], AF.Exp, scale=-1.0)

                    tt = t_pool.tile([128, F], BF16, tag="t")
                    nc.vector.tensor_tensor(tt[:], kb[:], ht[:], OP.mult)
                    ct = c_pool.tile([128, F], BF16, tag="c")
                    nc.vector.tensor_tensor(ct[:], tt[:], act[:], OP.mult)
                    at = a_pool.tile([128, F], BF16, tag="a")
                    nc.scalar.activation(at[:], ct[:], AF.Copy, bias=0.625, scale=-0.075)
                    bt = b_pool.tile([128, F], BF16, tag="b")
                    nc.vector.tensor_tensor(bt[:], ct[:], et[:], OP.mult)

                    wt = w_pool.tile([128, F], FP32, tag="w")
                    ci = o * NG + g
                    init = 0.0 if tci == 0 else carry[:, ci : ci + 1]
                    nc.vector.tensor_tensor_scan(wt[:], at[:], bt[:], init, OP.mult, OP.add)
                    if tci < TC - 1:
                        nc.scalar.copy(carry[:, ci : ci + 1], wt[:, F - 1 : F])

                    for s in range(F // 512):
                        nc.tensor.matmul(
                            ps[0:1, s * 512 : (s + 1) * 512],
                            ones[:],
                            wt[:, s * 512 : (s + 1) * 512],
                            start=(g == 0),
                            stop=(g == NG - 1),
                        )
                row = r_pool.tile([1, F], FP32, tag="row")
                nc.scalar.activation(row[:], ps[:], AF.Copy, bias=0.0, scale=0.075)
                nc.sync.dma_start(out[o : o + 1, t0 : t0 + F], row[:, :])

    nc.compile()
    return nc


def _build_nc_v3() -> bass.Bass:
    """v3: no GPSIMD (its SBUF-port sharing poisons DVE), ACT-heavy, bf16.

    Measured on HW: GPSIMD tensor_scalar = 36us/tile and inflates concurrent
    DVE tensor_tensor 5-10x via the shared POOL SBUF port. ACT runs exactly
    at (N+352)/1.2 ns with its own port. So:
      DVE: d = e-k (f32); act = clip(u)->bf16; t = k*h (mixed -> bf16);
           c = t*act (bf16 2x); A = -0.075c+0.625 (bf16 TS 4x);
           B = c*eb (bf16 2x); scan(A,B) -> w bf16
      ACT: lnd = Ln(d); h = Exp(-lnd) -> bf16; eb = e -> bf16 (Copy);
           psum evacuation with 0.075 scale; carry copies
      PE:  bf16 ones^T @ w -> psum accumulation over neuron groups
    """
    BF16 = mybir.dt.bfloat16
    nc = bacc.Bacc(
        "TRN2", target_bir_lowering=False, debug=False, num_devices=N_CORES
    )
    u = nc.dram_tensor("u", [NL, T], FP32, kind="ExternalInput")
    k = nc.dram_tensor("k", [NL, O, T], FP32, kind="ExternalInput")
    e = nc.dram_tensor("e", [NL, O, T], FP32, kind="ExternalInput")
    out = nc.dram_tensor("out", [O, T], FP32, kind="ExternalOutput")

    with tile.TileContext(nc) as tc, ExitStack() as ctx:
        preload = mybir.InstLoadActFuncSet(
            name=nc.get_next_instruction_name(), act_func_set_id=6, ins=[], outs=[]
        )
        nc.scalar.add_instruction(preload)

        const_pool = ctx.enter_context(tc.tile_pool(name="const", bufs=1))
        ones = const_pool.tile([128, 1], BF16)
        nc.vector.memset(ones[:], 1.0)
        carry = const_pool.tile([128, O * NG], FP32)

        u_pool = ctx.enter_context(tc.tile_pool(name="u", bufs=2))
        act_pool = ctx.enter_context(tc.tile_pool(name="act", bufs=NG + 1))
        k_pool = ctx.enter_context(tc.tile_pool(name="k", bufs=2))
        e_pool = ctx.enter_context(tc.tile_pool(name="e", bufs=2))
        eb_pool = ctx.enter_context(tc.tile_pool(name="eb", bufs=3))
        d_pool = ctx.enter_context(tc.tile_pool(name="d", bufs=3))
        l_pool = ctx.enter_context(tc.tile_pool(name="l", bufs=3))
        h_pool = ctx.enter_context(tc.tile_pool(name="h", bufs=3))
        t_pool = ctx.enter_context(tc.tile_pool(name="t", bufs=2))
        c_pool = ctx.enter_context(tc.tile_pool(name="c", bufs=2))
        a_pool = ctx.enter_context(tc.tile_pool(name="a", bufs=3))
        b_pool = ctx.enter_context(tc.tile_pool(name="b", bufs=3))
        w_pool = ctx.enter_context(tc.tile_pool(name="w", bufs=2))
        r_pool = ctx.enter_context(tc.tile_pool(name="r", bufs=2))
        ps_pool = ctx.enter_context(tc.tile_pool(name="ps", bufs=2, space="PSUM"))

        acts: dict[int, object] = {}

        for tci in range(TC):
            t0 = tci * F
            for o in range(O):
                ps = ps_pool.tile([1, F], FP32, tag="ps", name=f"ps{tci}_{o}")
                for g in range(NG):
                    p0 = g * 128
                    if o == 0:
                        ut = u_pool.tile([128, F], FP32, tag="u")
                        nc.sync.dma_start(ut[:], u[p0 : p0 + 128, t0 : t0 + F])
                        av = act_pool.tile([128, F], BF16, tag="act")
                        nc.vector.tensor_scalar(av[:], ut[:], 0.0, 1.0, OP.max, OP.min)
                        acts[g] = av
                    act = acts[g]

                    kt = k_pool.tile([128, F], FP32, tag="k")
                    nc.sync.dma_start(kt[:], k[p0 : p0 + 128, o, t0 : t0 + F])
                    et = e_pool.tile([128, F], FP32, tag="e")
                    nc.sync.dma_start(et[:], e[p0 : p0 + 128, o, t0 : t0 + F])

                    dt = d_pool.tile([128, F], FP32, tag="d")
                    nc.vector.tensor_tensor(dt[:], et[:], kt[:], OP.subtract)
                    eb = eb_pool.tile([128, F], BF16, tag="eb")
                    nc.scalar.copy(eb[:], et[:])
                    lnd = l_pool.tile([128, F], FP32, tag="lnd")
                    nc.scalar.activation(lnd[:], dt[:], AF.Ln)
                    ht = h_pool.tile([128, F], BF16, tag="h")
                    nc.scalar.activation(ht[:], lnd[:], AF.Exp, scale=-1.0)

                    tt = t_pool.tile([128, F], BF16, tag="t")
                    nc.vector.tensor_tensor(tt[:], kt[:], ht[:], OP.mult)
                    ct = c_pool.tile([128, F], BF16, tag="c")
                    nc.vector.tensor_tensor(ct[:], tt[:], act[:], OP.mult)
                    at = a_pool.tile([128, F], BF16, tag="a")
                    nc.vector.tensor_scalar(at[:], ct[:], -0.075, 0.625, OP.mult, OP.add)
                    bt = b_pool.tile([128, F], BF16, tag="b")
                    nc.vector.tensor_tensor(bt[:], ct[:], eb[:], OP.mult)

                    wt = w_pool.tile([128, F], BF16, tag="w")
                    ci = o * NG + g
                    init = 0.0 if tci == 0 else carry[:, ci : ci + 1]
                    nc.vector.tensor_tensor_scan(wt[:], at[:], bt[:], init, OP.mult, OP.add)
                    if tci < TC - 1:
                        nc.scalar.copy(carry[:, ci : ci + 1], wt[:, F - 1 : F])

                    for s in range(F // 512):
                        nc.tensor.matmul(
                            ps[0:1, s * 512 : (s + 1) * 512],
                            ones[:],
                            wt[:, s * 512 : (s + 1) * 512],
                            start=(g == 0),
                            stop=(g == NG - 1),
                        )
                row = r_pool.tile([1, F], FP32, tag="row")
                nc.scalar.activation(row[:], ps[:], AF.Copy, bias=0.0, scale=0.075)
                nc.sync.dma_start(out[o : o + 1, t0 : t0 + F], row[:, :])

    nc.compile()
    return nc


_NC_CACHE: list = []


def kernel(u_pre: np.ndarray, k_syn: np.ndarray, e_syn: np.ndarray) -> np.ndarray:
    if not _NC_CACHE:
        _NC_CACHE.append(_build_nc_v3())
    nc = _NC_CACHE[0]

    in_maps = []
    for i in range(N_CORES):
        lo, hi = i * NL, (i + 1) * NL
        in_maps.append(
            {
                "u": np.ascontiguousarray(u_pre[lo:hi, 0, :], dtype=np.float32),
                "k": np.ascontiguousarray(k_syn[lo:hi], dtype=np.float32),
                "e": np.ascontiguousarray(e_syn[lo:hi], dtype=np.float32),
            }
        )
    res = run_bass_kernel_spmd(nc, in_maps, list(range(N_CORES)))
    partials = np.stack([res.results[i]["out"] for i in range(N_CORES)])
    return partials.sum(axis=0, dtype=np.float32)



# revision 2
# speedup vs baseline: 1.2607x; 1.2607x over previous
"""Trainium2 Bass kernel for nn_NonSpikingOutput.

Reference semantics (N=4096 neurons, O=3 outputs, T=4096 steps):
    g = k/(e-k); act = clip(u, 0, 1); RK2 with i_syn frozen collapses to
        v_t = a_t * v_{t-1} + b_t
        a_t = 0.625 - 0.075*act*g,  b_t = 0.075*act*g*e = (0.625 - a_t)*e
    out[o, t] = sum_n v[n, o, t]

v4 design (from HW microbenchmarks):
  - Inputs are uploaded as bf16 (host truncation): halves HBM traffic and
    makes every DVE tensor_tensor eligible for the 2x perf mode.
  - 0.075 is folded into the ACT Exp bias: h = exp(-ln(e-k) + ln 0.075)
    = 0.075/(e-k), so c = act*k*h = 0.075*act*g, a = 0.625 - c (ACT Copy),
    b = c*e (DVE TT), and the scan yields v directly -- no rescale.
  - DVE carries only: clip (TS 4x), d=e-k, t=k*h, c=t*act, b=c*e (TT 2x),
    and the tensor_tensor_scan (2.15 ns/elem, dtype-independent).
  - ACT carries: Ln, Exp, the a affine, carry copies, psum evacuation.
  - PE reduces over neurons via ones^T @ w into a (1, F) psum row.
  - No GPSIMD (concurrent GPSIMD inflates DVE TT 4.5x via SBUF port sharing).

Sharding: neuron dim N split across 8 cores (512 each); host sums the
per-core (O, T) partials.
"""

import sys
from contextlib import ExitStack

import numpy as np

sys.path.insert(0, "/opt/trn_rl_repo")

import concourse.bass as bass
import concourse.tile as tile
from concourse import bacc, mybir
from concourse.bass_utils import run_bass_kernel_spmd

N_CORES = 8
N, O, T = 4096, 3, 4096
NL = N // N_CORES  # neurons per core
NG = NL // 128     # 128-partition neuron groups per core
F = 2048           # time-chunk (free dim) per tile
TC = T // F
FP32 = mybir.dt.float32
BF16 = mybir.dt.bfloat16
OP = mybir.AluOpType
AF = mybir.ActivationFunctionType

LN_0075 = float(np.log(0.075))  # Exp bias: exp(-lnd + ln 0.075) = 0.075/d


def _build_nc() -> bass.Bass:
    nc = bacc.Bacc(
        "TRN2", target_bir_lowering=False, debug=False, num_devices=N_CORES
    )
    u = nc.dram_tensor("u", [NL, T], BF16, kind="ExternalInput")
    k = nc.dram_tensor("k", [NL, O, T], BF16, kind="ExternalInput")
    e = nc.dram_tensor("e", [NL, O, T], BF16, kind="ExternalInput")
    out = nc.dram_tensor("out", [O, T], FP32, kind="ExternalOutput")

    with tile.TileContext(nc) as tc, ExitStack() as ctx:
        # Preload the ACT table set holding Ln+Exp+Copy (set 6) once.
        preload = mybir.InstLoadActFuncSet(
            name=nc.get_next_instruction_name(), act_func_set_id=6, ins=[], outs=[]
        )
        nc.scalar.add_instruction(preload)

        const_pool = ctx.enter_context(tc.tile_pool(name="const", bufs=1))
        ones = const_pool.tile([128, 1], BF16)
        nc.vector.memset(ones[:], 1.0)
        exp_bias = const_pool.tile([128, 1], FP32)
        nc.vector.memset(exp_bias[:], LN_0075)
        # one carry column per (o, g): column o*NG+g
        carry = const_pool.tile([128, O * NG], FP32)

        u_pool = ctx.enter_context(tc.tile_pool(name="u", bufs=2))
        act_pool = ctx.enter_context(tc.tile_pool(name="act", bufs=NG + 1))
        k_pool = ctx.enter_context(tc.tile_pool(name="k", bufs=3))
        e_pool = ctx.enter_context(tc.tile_pool(name="e", bufs=3))
        d_pool = ctx.enter_context(tc.tile_pool(name="d", bufs=2))
        l_pool = ctx.enter_context(tc.tile_pool(name="l", bufs=2))
        h_pool = ctx.enter_context(tc.tile_pool(name="h", bufs=2))
        t_pool = ctx.enter_context(tc.tile_pool(name="t", bufs=2))
        c_pool = ctx.enter_context(tc.tile_pool(name="c", bufs=2))
        a_pool = ctx.enter_context(tc.tile_pool(name="a", bufs=2))
        b_pool = ctx.enter_context(tc.tile_pool(name="b", bufs=2))
        w_pool = ctx.enter_context(tc.tile_pool(name="w", bufs=2))
        r_pool = ctx.enter_context(tc.tile_pool(name="r", bufs=2))
        ps_pool = ctx.enter_context(tc.tile_pool(name="ps", bufs=2, space="PSUM"))

        acts: dict[int, object] = {}

        for tci in range(TC):
            t0 = tci * F
            for o in range(O):
                ps = ps_pool.tile([1, F], FP32, tag="ps", name=f"ps{tci}_{o}")
                for g in range(NG):
                    p0 = g * 128
                    if o == 0:
                        ut = u_pool.tile([128, F], BF16, tag="u")
                        nc.sync.dma_start(ut[:], u[p0 : p0 + 128, t0 : t0 + F])
                        av = act_pool.tile([128, F], BF16, tag="act")
                        nc.vector.tensor_scalar(av[:], ut[:], 0.0, 1.0, OP.max, OP.min)
                        acts[g] = av
                    act = acts[g]

                    kt = k_pool.tile([128, F], BF16, tag="k")
                    nc.sync.dma_start(kt[:], k[p0 : p0 + 128, o, t0 : t0 + F])
                    et = e_pool.tile([128, F], BF16, tag="e")
                    nc.sync.dma_start(et[:], e[p0 : p0 + 128, o, t0 : t0 + F])

                    dt = d_pool.tile([128, F], BF16, tag="d")
                    nc.vector.tensor_tensor(dt[:], et[:], kt[:], OP.subtract)
                    lnd = l_pool.tile([128, F], FP32, tag="lnd")
                    nc.scalar.activation(lnd[:], dt[:], AF.Ln)
                    ht = h_pool.tile([128, F], BF16, tag="h")
                    nc.scalar.activation(
                        ht[:], lnd[:], AF.Exp, bias=exp_bias[:], scale=-1.0
                    )

                    tt = t_pool.tile([128, F], BF16, tag="t")
                    nc.vector.tensor_tensor(tt[:], kt[:], ht[:], OP.mult)
                    ct = c_pool.tile([128, F], BF16, tag="c")
                    nc.vector.tensor_tensor(ct[:], tt[:], act[:], OP.mult)
                    at = a_pool.tile([128, F], BF16, tag="a")
                    nc.scalar.activation(at[:], ct[:], AF.Copy, bias=0.625, scale=-1.0)
                    bt = b_pool.tile([128, F], BF16, tag="b")
                    nc.vector.tensor_tensor(bt[:], ct[:], et[:], OP.mult)

                    wt = w_pool.tile([128, F], BF16, tag="w")
                    ci = o * NG + g
                    init = 0.0 if tci == 0 else carry[:, ci : ci + 1]
                    nc.vector.tensor_tensor_scan(wt[:], at[:], bt[:], init, OP.mult, OP.add)
                    if tci < TC - 1:
                        nc.scalar.copy(carry[:, ci : ci + 1], wt[:, F - 1 : F])

                    for s in range(F // 512):
                        nc.tensor.matmul(
                            ps[0:1, s * 512 : (s + 1) * 512],
                            ones[:],
                            wt[:, s * 512 : (s + 1) * 512],
                            start=(g == 0),
                            stop=(g == NG - 1),
                        )
                row = r_pool.tile([1, F], FP32, tag="row")
                nc.scalar.copy(row[:], ps[:])
                nc.sync.dma_start(out[o : o + 1, t0 : t0 + F], row[:, :])

    nc.compile()
    return nc


_NC_CACHE: list = []


def _to_bf16(a: np.ndarray) -> np.ndarray:
    """Truncate f32 -> bf16 (round-to-nearest-even via numpy astype if
    ml_dtypes is available, else bit truncation)."""
    try:
        import ml_dtypes

        return np.ascontiguousarray(a.astype(ml_dtypes.bfloat16))
    except ImportError:
        b = np.ascontiguousarray(a, dtype=np.float32).view(np.uint32)
        # round-to-nearest-even on the high 16 bits
        b = ((b + 0x7FFF + ((b >> 16) & 1)) >> 16).astype(np.uint16)
        return b.view(np.dtype("uint16"))


def kernel(u_pre: np.ndarray, k_syn: np.ndarray, e_syn: np.ndarray) -> np.ndarray:
    if not _NC_CACHE:
        _NC_CACHE.append(_build_nc())
    nc = _NC_CACHE[0]

    in_maps = []
    for i in range(N_CORES):
        lo, hi = i * NL, (i + 1) * NL
        in_maps.append(
            {
                "u": _to_bf16(u_pre[lo:hi, 0, :]),
                "k": _to_bf16(k_syn[lo:hi]),
                "e": _to_bf16(e_syn[lo:hi]),
            }
        )
    res = run_bass_kernel_spmd(nc, in_maps, list(range(N_CORES)))
    partials = np.stack([res.results[i]["out"] for i in range(N_CORES)])
    return partials.sum(axis=0, dtype=np.float32)


# revision 4
# speedup vs baseline: 1.3527x; 1.0730x over previous
"""Trainium2 Bass kernel for nn_NonSpikingOutput.

Reference semantics (N=4096 neurons, O=3 outputs, T=4096 steps):
    g = k/(e-k); act = clip(u, 0, 1); RK2 with i_syn frozen collapses to
        v_t = a_t * v_{t-1} + b_t
        a_t = 0.625 - 0.075*act*g,  b_t = 0.075*act*g*e = (0.625 - a_t)*e
    out[o, t] = sum_n v[n, o, t]

v5 design (from HW microbenchmarks):
  - Inputs uploaded as bf16 (host truncation): halves HBM traffic and makes
    every DVE tensor_tensor eligible for the 2x perf mode.
  - d = e-k computed on the PE: psum_d = I@e + (-I)@k (identity stationaries
    uploaded as host constants). Removes one DVE TT per tile; ACT Ln reads
    the f32 psum directly.
  - 0.075 folded into the ACT Exp bias: h = exp(-ln(e-k) + ln 0.075)
    = 0.075/(e-k), so c = act*k*h, a = 0.625 - c (ACT Copy), b = c*e (DVE),
    and the scan yields v directly -- no rescale.
  - DVE carries only: clip (TS 4x), t=k*h, c=t*act, b=c*e (TT 2x bf16), scan.
  - ACT: Ln (from psum), Exp, the a affine, carry copies, psum evacuation.
  - PE: d subtraction + ones^T @ w neuron reduction.
  - PSUM: d tile (128,2048)f32 = 4 banks (bufs=1) + colsum row (1,2048)f32 =
    4 banks (bufs=1) -- exactly 8 banks.
  - No GPSIMD (concurrent GPSIMD inflates DVE TT 4.5x via SBUF port sharing).

Sharding: neuron dim N split across 8 cores (512 each); host sums the
per-core (O, T) partials.
"""

import sys
from contextlib import ExitStack

import numpy as np

sys.path.insert(0, "/opt/trn_rl_repo")

import concourse.bass as bass
import concourse.tile as tile
from concourse import bacc, mybir
from concourse.bass_utils import run_bass_kernel_spmd

N_CORES = 8
N, O, T = 4096, 3, 4096
NL = N // N_CORES  # neurons per core
NG = NL // 128     # 128-partition neuron groups per core
F = 2048           # time-chunk (free dim) per tile
TC = T // F
FP32 = mybir.dt.float32
BF16 = mybir.dt.bfloat16
OP = mybir.AluOpType
AF = mybir.ActivationFunctionType

LN_0075 = float(np.log(0.075))  # Exp bias: exp(-lnd + ln 0.075) = 0.075/d


def _build_nc() -> bass.Bass:
    nc = bacc.Bacc(
        "TRN2", target_bir_lowering=False, debug=False, num_devices=N_CORES
    )
    u = nc.dram_tensor("u", [NL, T], BF16, kind="ExternalInput")
    k = nc.dram_tensor("k", [NL, O, T], BF16, kind="ExternalInput")
    e = nc.dram_tensor("e", [NL, O, T], BF16, kind="ExternalInput")
    ident_d = nc.dram_tensor("ident", [128, 128], BF16, kind="ExternalInput")
    nident_d = nc.dram_tensor("nident", [128, 128], BF16, kind="ExternalInput")
    out = nc.dram_tensor("out", [O, T], FP32, kind="ExternalOutput")

    with tile.TileContext(nc) as tc, ExitStack() as ctx:
        # Preload the ACT table set holding Ln+Exp+Copy (set 6) once.
        preload = mybir.InstLoadActFuncSet(
            name=nc.get_next_instruction_name(), act_func_set_id=6, ins=[], outs=[]
        )
        nc.scalar.add_instruction(preload)

        const_pool = ctx.enter_context(tc.tile_pool(name="const", bufs=1))
        ones = const_pool.tile([128, 1], BF16)
        nc.vector.memset(ones[:], 1.0)
        exp_bias = const_pool.tile([128, 1], FP32)
        nc.vector.memset(exp_bias[:], LN_0075)
        ident = const_pool.tile([128, 128], BF16)
        nc.sync.dma_start(ident[:], ident_d[:, :])
        nident = const_pool.tile([128, 128], BF16)
        nc.sync.dma_start(nident[:], nident_d[:, :])
        # one carry column per (o, g): column o*NG+g
        carry = const_pool.tile([128, O * NG], FP32)

        u_pool = ctx.enter_context(tc.tile_pool(name="u", bufs=2))
        act_pool = ctx.enter_context(tc.tile_pool(name="act", bufs=NG + 1))
        k_pool = ctx.enter_context(tc.tile_pool(name="k", bufs=3))
        e_pool = ctx.enter_context(tc.tile_pool(name="e", bufs=3))
        l_pool = ctx.enter_context(tc.tile_pool(name="l", bufs=2))
        h_pool = ctx.enter_context(tc.tile_pool(name="h", bufs=2))
        t_pool = ctx.enter_context(tc.tile_pool(name="t", bufs=2))
        c_pool = ctx.enter_context(tc.tile_pool(name="c", bufs=2))
        a_pool = ctx.enter_context(tc.tile_pool(name="a", bufs=2))
        b_pool = ctx.enter_context(tc.tile_pool(name="b", bufs=2))
        w_pool = ctx.enter_context(tc.tile_pool(name="w", bufs=2))
        r_pool = ctx.enter_context(tc.tile_pool(name="r", bufs=2))
        ps_pool = ctx.enter_context(tc.tile_pool(name="ps", bufs=1, space="PSUM"))
        d_pool = ctx.enter_context(tc.tile_pool(name="d", bufs=1, space="PSUM"))

        acts: dict[int, object] = {}

        for tci in range(TC):
            t0 = tci * F
            for o in range(O):
                ps = ps_pool.tile([1, F], FP32, tag="ps", name=f"ps{tci}_{o}")
                for g in range(NG):
                    p0 = g * 128
                    if o == 0:
                        ut = u_pool.tile([128, F], BF16, tag="u")
                        nc.sync.dma_start(ut[:], u[p0 : p0 + 128, t0 : t0 + F])
                        av = act_pool.tile([128, F], BF16, tag="act")
                        nc.vector.tensor_scalar(av[:], ut[:], 0.0, 1.0, OP.max, OP.min)
                        acts[g] = av
                    act = acts[g]

                    kt = k_pool.tile([128, F], BF16, tag="k")
                    nc.sync.dma_start(kt[:], k[p0 : p0 + 128, o, t0 : t0 + F])
                    et = e_pool.tile([128, F], BF16, tag="e")
                    nc.sync.dma_start(et[:], e[p0 : p0 + 128, o, t0 : t0 + F])

                    # d = e - k on the PE: per 512-chunk, I@e then (-I)@k
                    dps = d_pool.tile([128, F], FP32, tag="d", name=f"d{tci}_{o}_{g}")
                    for s in range(F // 512):
                        sl = slice(s * 512, (s + 1) * 512)
                        nc.tensor.matmul(
                            dps[:, sl], ident[:], et[:, sl], start=True, stop=False
                        )
                        nc.tensor.matmul(
                            dps[:, sl], nident[:], kt[:, sl], start=False, stop=True
                        )

                    lnd = l_pool.tile([128, F], FP32, tag="lnd")
                    nc.scalar.activation(lnd[:], dps[:], AF.Ln)
                    ht = h_pool.tile([128, F], BF16, tag="h")
                    nc.scalar.activation(
                        ht[:], lnd[:], AF.Exp, bias=exp_bias[:], scale=-1.0
                    )

                    tt = t_pool.tile([128, F], BF16, tag="t")
                    nc.vector.tensor_tensor(tt[:], kt[:], ht[:], OP.mult)
                    ct = c_pool.tile([128, F], BF16, tag="c")
                    nc.vector.tensor_tensor(ct[:], tt[:], act[:], OP.mult)
                    at = a_pool.tile([128, F], BF16, tag="a")
                    nc.scalar.activation(at[:], ct[:], AF.Copy, bias=0.625, scale=-1.0)
                    bt = b_pool.tile([128, F], BF16, tag="b")
                    nc.vector.tensor_tensor(bt[:], ct[:], et[:], OP.mult)

                    wt = w_pool.tile([128, F], BF16, tag="w")
                    ci = o * NG + g
                    init = 0.0 if tci == 0 else carry[:, ci : ci + 1]
                    nc.vector.tensor_tensor_scan(wt[:], at[:], bt[:], init, OP.mult, OP.add)
                    if tci < TC - 1:
                        nc.scalar.copy(carry[:, ci : ci + 1], wt[:, F - 1 : F])

                    for s in range(F // 512):
                        nc.tensor.matmul(
                            ps[0:1, s * 512 : (s + 1) * 512],
                            ones[:],
                            wt[:, s * 512 : (s + 1) * 512],
                            start=(g == 0),
                            stop=(g == NG - 1),
                        )
                row = r_pool.tile([1, F], FP32, tag="row")
                nc.scalar.copy(row[:], ps[:])
                nc.sync.dma_start(out[o : o + 1, t0 : t0 + F], row[:, :])

    nc.compile()
    return nc


_NC_CACHE: list = []


def _to_bf16(a: np.ndarray) -> np.ndarray:
    import ml_dtypes

    return np.ascontiguousarray(a.astype(ml_dtypes.bfloat16))


def build_in_maps(u_pre: np.ndarray, k_syn: np.ndarray, e_syn: np.ndarray) -> list:
    import ml_dtypes

    eye = np.eye(128, dtype=ml_dtypes.bfloat16)
    neye = (-np.eye(128)).astype(ml_dtypes.bfloat16)
    in_maps = []
    for i in range(N_CORES):
        lo, hi = i * NL, (i + 1) * NL
        in_maps.append(
            {
                "u": _to_bf16(u_pre[lo:hi, 0, :]),
                "k": _to_bf16(k_syn[lo:hi]),
                "e": _to_bf16(e_syn[lo:hi]),
                "ident": eye,
                "nident": neye,
            }
        )
    return in_maps


def kernel(u_pre: np.ndarray, k_syn: np.ndarray, e_syn: np.ndarray) -> np.ndarray:
    if not _NC_CACHE:
        _NC_CACHE.append(_build_nc())
    nc = _NC_CACHE[0]

    in_maps = build_in_maps(u_pre, k_syn, e_syn)
    res = run_bass_kernel_spmd(nc, in_maps, list(range(N_CORES)))
    partials = np.stack([res.results[i]["out"] for i in range(N_CORES)])
    return partials.sum(axis=0, dtype=np.float32)
